# revision 15
# baseline (speedup 1.0000x reference)
"""Fused multi-head cross-attention with relation branch, sharded over 8 NeuronCores.

Sharding: data-parallel over batch (4) x tensor-parallel over head halves (2).
Core c handles batch c//2, heads [8*(c%2), 8*(c%2)+8). Each core computes its
partial output projection; the host sums the two partials per batch and adds bo.

v2 schedule: dc-granular interleave so the scalar (exp) engine starts ~16us in
instead of ~57us, normalize chains read PV psum directly (no staging copies),
denominator rows DMA straight from PSUM to the DRAM reshape bounce, odd heads
carry their softmax-denominator ones-column FIRST so their PV accumulator sits
at psum partitions 63..127 and the normalized add writes xf rows 64..127 with
no partition-shift DMA, and the output projection DMAs straight from PSUM.

Emission order (tensor stream):
  for dc: q-proj(dc), k-proj(dc), rk-proj(dc), scores(dc, lqh=0, vis|rel)
  v-proj, rv-proj
  for dc: PV(lqh0, dc)+chain, scores(dc, lqh=1, vis|rel)
  PV(lqh1, 0)+chain, outproj(lqh0), PV(lqh1, 1..3)+chain, outproj(lqh1, wide)

PSUM: one pool, tags "prj" (2 slots x 2 banks: projections, PV accumulators,
wide outproj) + "spool" (2 slots x 2 banks: score tiles, outproj lqh0) = 8 banks.
"""

import math

import numpy as np

B, LQ, LK, D, H = 4, 1024, 1024, 1024, 16
DK = D // H
SCALE = 1.0 / math.sqrt(DK)
N_CORES = 8
HD = D // 2  # local dims per core (8 heads * 64)
# Keys are compacted host-side: only unmasked keys are shipped (padded to LKP
# with dummy rows whose mask bias is -1e9, so exp()=0 -> exact same math).
LKP = 640
NM = LKP // 128  # lk chunks

_CACHE = {}


def _build_program(lkp=LKP):
    import concourse.bacc as bacc
    import concourse.mybir as mybir
    import concourse.tile as tile

    LKP = lkp
    NM = LKP // 128

    f32 = mybir.dt.float32
    bf16 = mybir.dt.bfloat16
    Exp = mybir.ActivationFunctionType.Exp
    Add = mybir.AluOpType.add
    Mult = mybir.AluOpType.mult

    nc = bacc.Bacc(
        "TRN2",
        target_bir_lowering=False,
        debug=False,
        enable_asserts=False,
        num_devices=N_CORES,
    )

    # DRAM I/O (per-core shapes; host shards/pre-transposes/casts).
    xqT = nc.dram_tensor("xqT", [D, LQ], bf16, kind="ExternalInput").ap()
    xkT = nc.dram_tensor("xkT", [D, LKP], bf16, kind="ExternalInput").ap()
    xrT = nc.dram_tensor("xrT", [D, LKP], bf16, kind="ExternalInput").ap()
    xvT = nc.dram_tensor("xvT", [D, LKP], bf16, kind="ExternalInput").ap()
    wqT = nc.dram_tensor("wqT", [D, HD], bf16, kind="ExternalInput").ap()
    wkT = nc.dram_tensor("wkT", [D, HD], bf16, kind="ExternalInput").ap()
    wrkT = nc.dram_tensor("wrkT", [D, HD], bf16, kind="ExternalInput").ap()
    wvT = nc.dram_tensor("wvT", [D, HD], bf16, kind="ExternalInput").ap()
    wrvT = nc.dram_tensor("wrvT", [D, HD], bf16, kind="ExternalInput").ap()
    woT = nc.dram_tensor("woT", [HD, D], bf16, kind="ExternalInput").ap()
    bq_pc = nc.dram_tensor("bq_pc", [128, 4], f32, kind="ExternalInput").ap()
    bk_pc = nc.dram_tensor("bk_pc", [128, 4], f32, kind="ExternalInput").ap()
    brk_pc = nc.dram_tensor("brk_pc", [128, 4], f32, kind="ExternalInput").ap()
    bv_bc = nc.dram_tensor("bv_bc", [128, HD], f32, kind="ExternalInput").ap()
    brv_bc = nc.dram_tensor("brv_bc", [128, HD], f32, kind="ExternalInput").ap()
    maskb = nc.dram_tensor("maskb", [128, NM], f32, kind="ExternalInput").ap()
    yT = nc.dram_tensor("yT", [D, LQ], f32, kind="ExternalOutput").ap()
    scr2 = nc.dram_tensor("scr2", [8, 2048], bf16, kind="Internal").ap()

    with tile.TileContext(nc) as tc:
        from contextlib import ExitStack

        with ExitStack() as ctx:
            # Persistent SBUF tensors.
            persist = ctx.enter_context(tc.tile_pool(name="persist", bufs=1))
            qT_sb = persist.tile([128, 4 * LQ], bf16, tag="qT")
            kT_sb = persist.tile([128, 4 * LKP], bf16, tag="kT")
            rkT_sb = persist.tile([128, 4 * LKP], bf16, tag="rkT")
            v_sb = persist.tile([128, NM * 8 * 65], bf16, tag="v")
            rv_sb = persist.tile([128, NM * 8 * 65], bf16, tag="rv")
            xf_sb = persist.tile([128, 4 * LQ], bf16, tag="xf")
            maskb_sb = persist.tile([128, NM], f32, tag="maskb")
            bq_sb = persist.tile([128, 4], f32, tag="bq")
            bk_sb = persist.tile([128, 4], f32, tag="bk")
            brk_sb = persist.tile([128, 4], f32, tag="brk")
            bv_sb = persist.tile([128, HD], f32, tag="bv")
            brv_sb = persist.tile([128, HD], f32, tag="brv")

            nc.sync.dma_start(out=maskb_sb[:], in_=maskb)
            nc.sync.dma_start(out=bq_sb[:], in_=bq_pc)
            nc.sync.dma_start(out=bk_sb[:], in_=bk_pc)
            nc.sync.dma_start(out=brk_sb[:], in_=brk_pc)
            nc.sync.dma_start(out=bv_sb[:], in_=bv_bc)
            nc.sync.dma_start(out=brv_sb[:], in_=brv_bc)

            # v/rv: [128, m, hp, two, 65], each head [v|1] (ones column last
            # accumulates the softmax denominator in the PV matmul's row 64).
            v5 = v_sb[:].rearrange("p (m hp t c) -> p m hp t c", m=NM, hp=4, t=2, c=65)
            rv5 = rv_sb[:].rearrange("p (m hp t c) -> p m hp t c", m=NM, hp=4, t=2, c=65)
            for vv in (v5, rv5):
                nc.vector.memset(vv[:, :, :, :, 64:65], 1.0)

            # Input tiles. Tag reuse: xq slots -> xv; wq -> wrv; wk -> wv;
            # wrk slots (sized for wo) -> wo.
            inp = ctx.enter_context(tc.tile_pool(name="inp", bufs=8))
            psum = ctx.enter_context(tc.tile_pool(name="psum", bufs=2, space="PSUM"))
            ppool = ctx.enter_context(tc.tile_pool(name="ppool", bufs=38))
            sgp = ctx.enter_context(tc.tile_pool(name="sgp", bufs=2))
            bcp = ctx.enter_context(tc.tile_pool(name="bcp", bufs=6))
            tp = ctx.enter_context(tc.tile_pool(name="tp", bufs=4))
            ysb = ctx.enter_context(tc.tile_pool(name="ysb", bufs=2))

            # ---- input DMA issue (sync queue: q,k,rk in need-order) ----
            xq_ch, wq_ch = [], []
            for k in range(8):
                w = inp.tile([128, HD], bf16, tag="w1", name=f"wq{k}")
                nc.sync.dma_start(out=w[:], in_=wqT[128 * k : 128 * k + 128, :])
                wq_ch.append(w)
                t = inp.tile([128, LQ], bf16, tag="xbig", name=f"xq{k}")
                nc.sync.dma_start(out=t[:], in_=xqT[128 * k : 128 * k + 128, :])
                xq_ch.append(t)
            xk_ch, wk_ch = [], []
            for k in range(8):
                w = inp.tile([128, HD], bf16, tag="w2", name=f"wk{k}")
                nc.sync.dma_start(out=w[:], in_=wkT[128 * k : 128 * k + 128, :])
                wk_ch.append(w)
                t = inp.tile([128, LKP], bf16, tag="xk", name=f"xk{k}")
                nc.sync.dma_start(out=t[:], in_=xkT[128 * k : 128 * k + 128, :])
                xk_ch.append(t)
            xr_ch, wrk_ch = [], []
            for k in range(8):
                w = inp.tile([128, 1024], bf16, tag="w3", name=f"wrk{k}")
                nc.sync.dma_start(out=w[:, 0:HD], in_=wrkT[128 * k : 128 * k + 128, :])
                wrk_ch.append(w)
                t = inp.tile([128, LKP], bf16, tag="xr", name=f"xr{k}")
                nc.sync.dma_start(out=t[:], in_=xrT[128 * k : 128 * k + 128, :])
                xr_ch.append(t)
            # v inputs + wv/wrv/wo on the gpsimd queue (slot-reuse WAR waits
            # live on that queue, off the critical sync queue).
            xv_ch, wv_ch, wrv_ch = [], [], []
            for k in range(8):
                w = inp.tile([128, HD], bf16, tag="w2", name=f"wv{k}")
                nc.gpsimd.dma_start(out=w[:], in_=wvT[128 * k : 128 * k + 128, :])
                wv_ch.append(w)
                t = inp.tile([128, LQ], bf16, tag="xbig", name=f"xv{k}")
                nc.gpsimd.dma_start(out=t[:, 0:LKP], in_=xvT[128 * k : 128 * k + 128, :])
                xv_ch.append(t)
            for k in range(8):
                w = inp.tile([128, HD], bf16, tag="w1", name=f"wrv{k}")
                nc.gpsimd.dma_start(out=w[:], in_=wrvT[128 * k : 128 * k + 128, :])
                wrv_ch.append(w)
            wo_ch = []
            for dc in range(4):
                w = inp.tile([128, 1024], bf16, tag="w3", name=f"wo{dc}")
                nc.gpsimd.dma_start(out=w[:], in_=woT[128 * dc : 128 * dc + 128, :])
                wo_ch.append(w)

            p_tiles = {}

            def emit_scores(dc, lqh, br):
                kt = kT_sb if br == 0 else rkT_sb
                qsl = slice(1024 * dc + 512 * lqh, 1024 * dc + 512 * lqh + 512)
                for m in range(NM):
                    ksl = slice(LKP * dc + 128 * m, LKP * dc + 128 * m + 128)
                    s = psum.tile([128, 1024], f32, tag="spool", name="s")
                    nc.tensor.matmul(s[:, 0:512], kt[0:64, ksl], qT_sb[0:64, qsl])
                    nc.tensor.matmul(s[:, 512:1024], kt[64:128, ksl], qT_sb[64:128, qsl])
                    p = ppool.tile([128, 1024], bf16, tag="ppool", name="p")
                    nc.scalar.activation(
                        p[:], s[:], Exp, bias=maskb_sb[:, m : m + 1], scale=SCALE
                    )
                    p_tiles[(lqh, dc, m, br)] = p

            def proj_T(dc, xch, wch, b_sb, out_sb, LL):
                # Transposed projection chunk dc -> out_sb[:, LL*dc : LL*dc+LL]
                ps = psum.tile([128, 1024], f32, tag="prj", name="ps")
                nsl = [slice(a, min(a + 512, LL)) for a in range(0, LL, 512)]
                for k in range(8):
                    for sl in nsl:
                        nc.tensor.matmul(
                            ps[:, sl],
                            wch[k][:, 128 * dc : 128 * dc + 128],
                            xch[k][:, sl],
                            start=(k == 0),
                            stop=(k == 7),
                        )
                nc.vector.tensor_scalar(
                    out=out_sb[:, LL * dc : LL * dc + LL],
                    in0=ps[:, 0:LL],
                    scalar1=b_sb[:, dc : dc + 1],
                    scalar2=None,
                    op0=Add,
                )

            # ---- phase A: q/k/rk projections + lqh0 scores, dc-granular ----
            for dc in range(4):
                proj_T(dc, xq_ch, wq_ch, bq_sb, qT_sb, LQ)
                proj_T(dc, xk_ch, wk_ch, bk_sb, kT_sb, LKP)
                proj_T(dc, xr_ch, wrk_ch, brk_sb, rkT_sb, LKP)
                emit_scores(dc, 0, 0)
                emit_scores(dc, 0, 1)

            # ---- phase B: v/rv projections (natural orientation) ----
            bvv = bv_sb[:].rearrange("p (hp t c) -> p hp t c", hp=4, t=2, c=64)
            brvv = brv_sb[:].rearrange("p (hp t c) -> p hp t c", hp=4, t=2, c=64)
            for xch, wch, bview, out5 in ((xv_ch, wv_ch, bvv, v5), (xr_ch, wrv_ch, brvv, rv5)):
                for m in range(NM):
                    ps = psum.tile([128, 1024], f32, tag="prj", name="psv")
                    for k in range(8):
                        nc.tensor.matmul(
                            ps[:, 0:512],
                            xch[k][:, 128 * m : 128 * m + 128],
                            wch[k][:],
                            start=(k == 0),
                            stop=(k == 7),
                        )
                    pv = ps[:, 0:512].rearrange("p (hp t c) -> p hp t c", hp=4, t=2, c=64)
                    nc.vector.tensor_tensor(
                        out=out5[:, m, :, :, 0:64], in0=pv[:], in1=bview[:], op=Add
                    )

            def emit_pv_chain(lqh, dc):
                # PV: two psum tiles (br), hs0 at [0:65, 0:512] and hs1 at
                # [0:65, 512:1024]; row 64 accumulates the denominator.
                xa = []
                for br in range(2):
                    a = psum.tile([128, 1024], f32, tag="prj", name=f"xa{br}")
                    vv = v5 if br == 0 else rv5
                    for m in range(NM):
                        pt = p_tiles[(lqh, dc, m, br)]
                        for hs in range(2):
                            nc.tensor.matmul(
                                a[0:65, 512 * hs : 512 * hs + 512],
                                vv[:, m, dc, hs, 0:65],
                                pt[:, 512 * hs : 512 * hs + 512],
                                start=(m == 0),
                                stop=(m == NM - 1),
                            )
                    xa.append(a)
                # Reciprocal straight off the psum denominator rows (DVE reads
                # PSUM; cost is free-size based so the single-lane shape is
                # fine), then SBUF-source partition-broadcast.
                it = 2 * dc + lqh
                for br in range(2):
                    di = sgp.tile([1, 1024], bf16, tag="dinv", name=f"dinv{br}")
                    with nc.allow_low_precision(reason="bf16 1/denom, ~0.4% rel"):
                        nc.vector.reciprocal(di[:], xa[br][64:65, :])
                    nc.sync.dma_start(
                        out=scr2[it, 1024 * br : 1024 * br + 1024], in_=di[:]
                    )
                bcs = []
                # scr2 row layout: [bcv0 | bcv1 | bcr0 | bcr1] = (br, hs)
                for j, (br, hs) in enumerate(((0, 0), (1, 0), (0, 1), (1, 1))):
                    off = 1024 * br + 512 * hs
                    bc = bcp.tile([64, 512], bf16, tag="bcp", name=f"bc{j}")
                    nc.gpsimd.dma_start(
                        out=bc[:],
                        in_=scr2[it : it + 1, off : off + 512]
                        .partition_broadcast(64)[:, 0, :],
                    )
                    bcs.append(bc)
                xfsl = slice(1024 * dc + 512 * lqh, 1024 * dc + 512 * lqh + 512)
                t1 = tp.tile([64, 512], bf16, tag="tp", name="t1")
                t2 = tp.tile([64, 512], bf16, tag="tp", name="t2")
                nc.vector.tensor_tensor(
                    out=t1[:], in0=xa[0][0:64, 0:512], in1=bcs[0][:], op=Mult
                )
                nc.vector.tensor_tensor(
                    out=t2[:], in0=xa[1][0:64, 0:512], in1=bcs[1][:], op=Mult
                )
                nc.vector.tensor_tensor(
                    out=xf_sb[0:64, xfsl], in0=t1[:], in1=t2[:], op=Add
                )
                t3 = tp.tile([64, 512], bf16, tag="tp", name="t3")
                t4 = tp.tile([64, 512], bf16, tag="tp", name="t4")
                nc.vector.tensor_tensor(
                    out=t3[:], in0=xa[0][0:64, 512:1024], in1=bcs[2][:], op=Mult
                )
                nc.vector.tensor_tensor(
                    out=t4[:], in0=xa[1][0:64, 512:1024], in1=bcs[3][:], op=Mult
                )
                thi = tp.tile([64, 512], bf16, tag="thi", name="thi", bufs=2)
                nc.vector.tensor_tensor(out=thi[:], in0=t3[:], in1=t4[:], op=Add)
                nc.sync.dma_start(out=xf_sb[64:128, xfsl], in_=thi[:])

            # ---- phase C: lqh1 scores + PV(lqh0) ----
            for dc in range(4):
                emit_pv_chain(0, dc)
                emit_scores(dc, 1, 0)
                emit_scores(dc, 1, 1)

            def emit_y_out(reg, ot, lqh):
                # PSUM -> SBUF copy on the scalar engine (idle once exp is
                # done, which is before any outproj runs), then DMA out.
                y = ysb.tile([128, 512], f32, tag="ysb", name="y")
                nc.scalar.copy(y[:], reg)
                nc.sync.dma_start(
                    out=yT[128 * ot : 128 * ot + 128, 512 * lqh : 512 * lqh + 512],
                    in_=y[:],
                )

            def emit_outproj(lqh, wide):
                if not wide:
                    for pair in range(4):
                        yt2 = psum.tile([128, 1024], f32, tag="spool", name="yt2")
                        for half in range(2):
                            ot = 2 * pair + half
                            reg = yt2[:, 512 * half : 512 * half + 512]
                            for dc in range(4):
                                nc.tensor.matmul(
                                    reg,
                                    wo_ch[dc][:, 128 * ot : 128 * ot + 128],
                                    xf_sb[:, 1024 * dc + 512 * lqh : 1024 * dc + 512 * lqh + 512],
                                    start=(dc == 0),
                                    stop=(dc == 3),
                                )
                            emit_y_out(reg, ot, lqh)
                    return
                # Wide: 8 accumulators (2 prj + 2 spool slots, 2 halves each),
                # dc-outer so only the last dim-chunk waits on the last chain.
                regs = []
                for i in range(4):
                    tag = "prj" if i < 2 else "spool"
                    w2 = psum.tile([128, 1024], f32, tag=tag, name=f"yw{i}")
                    regs.append(w2[:, 0:512])
                    regs.append(w2[:, 512:1024])
                for dc in range(4):
                    for ot in range(8):
                        nc.tensor.matmul(
                            regs[ot],
                            wo_ch[dc][:, 128 * ot : 128 * ot + 128],
                            xf_sb[:, 1024 * dc + 512 * lqh : 1024 * dc + 512 * lqh + 512],
                            start=(dc == 0),
                            stop=(dc == 3),
                        )
                for ot in range(8):
                    emit_y_out(regs[ot], ot, lqh)

            # ---- phase D: PV(lqh1) + outproj ----
            emit_pv_chain(1, 0)
            emit_outproj(0, wide=False)
            emit_pv_chain(1, 1)
            emit_pv_chain(1, 2)
            emit_pv_chain(1, 3)
            emit_outproj(1, wide=True)

    nc.compile()
    return nc


def _get_program(lkp=LKP):
    if lkp not in _CACHE:
        _CACHE[lkp] = _build_program(lkp)
    return _CACHE[lkp]


def _cast_bf16(arr):
    import ml_dtypes

    return np.ascontiguousarray(arr.astype(ml_dtypes.bfloat16))


def _shard_inputs(inputs, lkp=LKP):
    q = np.ascontiguousarray(inputs["query"], dtype=np.float32)
    k = np.ascontiguousarray(inputs["key"], dtype=np.float32)
    v = np.ascontiguousarray(inputs["value"], dtype=np.float32)
    wr = np.ascontiguousarray(inputs["weak_rela"], dtype=np.float32)
    mask = np.asarray(inputs["mask"])

    in_maps = []
    for c in range(N_CORES):
        b, hh = divmod(c, 2)
        hsl = slice(HD * hh, HD * hh + HD)
        idx = np.nonzero(mask[b, 0])[0]
        nv = len(idx)
        assert nv <= lkp
        pidx = np.concatenate([idx, np.zeros(lkp - nv, dtype=idx.dtype)])
        bias = np.full(lkp, -1.0e9, np.float32)
        bias[:nv] = 0.0
        mb = np.ascontiguousarray(bias.reshape(lkp // 128, 128).T)
        kc, vc, wrc = k[b][pidx], v[b][pidx], wr[b][pidx]
        m = {
            "xqT": _cast_bf16(q[b].T),
            "xkT": _cast_bf16(kc.T),
            "xrT": _cast_bf16(wrc.T),
            "xvT": _cast_bf16(vc.T),
            "wqT": _cast_bf16(np.asarray(inputs["Wq"])[hsl, :].T),
            "wkT": _cast_bf16(np.asarray(inputs["Wk"])[hsl, :].T),
            "wrkT": _cast_bf16(np.asarray(inputs["Wrk"])[hsl, :].T),
            "wvT": _cast_bf16(np.asarray(inputs["Wv"])[hsl, :].T),
            "wrvT": _cast_bf16(np.asarray(inputs["Wrv"])[hsl, :].T),
            "woT": _cast_bf16(np.asarray(inputs["Wo"])[:, hsl].T),
            "bq_pc": np.asarray(inputs["bq"][hsl]).reshape(4, 128).T.astype(np.float32),
            "bk_pc": np.asarray(inputs["bk"][hsl]).reshape(4, 128).T.astype(np.float32),
            "brk_pc": np.asarray(inputs["brk"][hsl])
            .reshape(4, 128)
            .T.astype(np.float32),
            "bv_bc": np.broadcast_to(inputs["bv"][hsl], (128, HD)).astype(np.float32),
            "brv_bc": np.broadcast_to(inputs["brv"][hsl], (128, HD)).astype(np.float32),
            "maskb": mb,
        }
        in_maps.append({k2: np.ascontiguousarray(v2) for k2, v2 in m.items()})
    return in_maps


def run_on_hw(inputs, trace=False, **kw):
    from concourse.bass_utils import run_bass_kernel_spmd

    mask = np.asarray(inputs["mask"])
    max_valid = max(int(mask[b, 0].sum()) for b in range(B))
    lkp = max(LKP, ((max_valid + 127) // 128) * 128)
    nc = _get_program(lkp)
    in_maps = _shard_inputs(inputs, lkp)
    res = run_bass_kernel_spmd(
        nc, in_maps, core_ids=list(range(N_CORES)), trace=trace, **kw
    )
    bo = np.asarray(inputs["bo"], dtype=np.float32)
    outs = []
    for b in range(B):
        yt = res.results[2 * b]["yT"] + res.results[2 * b + 1]["yT"]
        outs.append(yt.T + bo)
    out = np.stack(outs).astype(np.float32)
    return out, res


def kernel(**inputs):
    out, _ = run_on_hw(inputs)
    return out


# revision 17
# speedup vs baseline: 1.4306x; 1.4306x over previous
"""Fused multi-head cross-attention with relation branch, sharded over 8 NeuronCores.

Sharding: data-parallel over batch (4) x tensor-parallel over head halves (2).
Core c handles batch c//2, heads [8*(c%2), 8*(c%2)+8). Each core computes its
partial output projection; the host sums the two partials per batch and adds bo.

v2 schedule: dc-granular interleave so the scalar (exp) engine starts ~16us in
instead of ~57us, normalize chains read PV psum directly (no staging copies),
denominator rows DMA straight from PSUM to the DRAM reshape bounce, odd heads
carry their softmax-denominator ones-column FIRST so their PV accumulator sits
at psum partitions 63..127 and the normalized add writes xf rows 64..127 with
no partition-shift DMA, and the output projection DMAs straight from PSUM.

Emission order (tensor stream):
  for dc: q-proj(dc), k-proj(dc), rk-proj(dc), scores(dc, lqh=0, vis|rel)
  v-proj, rv-proj
  for dc: PV(lqh0, dc)+chain, scores(dc, lqh=1, vis|rel)
  PV(lqh1, 0)+chain, outproj(lqh0), PV(lqh1, 1..3)+chain, outproj(lqh1, wide)

PSUM: one pool, tags "prj" (2 slots x 2 banks: projections, PV accumulators,
wide outproj) + "spool" (2 slots x 2 banks: score tiles, outproj lqh0) = 8 banks.
"""

import math

import numpy as np

B, LQ, LK, D, H = 4, 1024, 1024, 1024, 16
DK = D // H
SCALE = 1.0 / math.sqrt(DK)
N_CORES = 8
HD = D // 2  # local dims per core (8 heads * 64)
# Keys are compacted host-side: only unmasked keys are shipped (padded to LKP
# with dummy rows whose mask bias is -1e9, so exp()=0 -> exact same math).
LKP = 640
NM = LKP // 128  # lk chunks

_CACHE = {}


def _build_program(lkp=LKP):
    import concourse.bacc as bacc
    import concourse.mybir as mybir
    import concourse.tile as tile

    LKP = lkp
    NM = LKP // 128

    f32 = mybir.dt.float32
    bf16 = mybir.dt.bfloat16
    Exp = mybir.ActivationFunctionType.Exp
    Add = mybir.AluOpType.add
    Mult = mybir.AluOpType.mult

    nc = bacc.Bacc(
        "TRN2",
        target_bir_lowering=False,
        debug=False,
        enable_asserts=False,
        num_devices=N_CORES,
    )

    # DRAM I/O (per-core shapes; host shards/pre-transposes/casts).
    xqT = nc.dram_tensor("xqT", [D, LQ], bf16, kind="ExternalInput").ap()
    xkT = nc.dram_tensor("xkT", [D, LKP], bf16, kind="ExternalInput").ap()
    xrT = nc.dram_tensor("xrT", [D, LKP], bf16, kind="ExternalInput").ap()
    xvT = nc.dram_tensor("xvT", [D, LKP], bf16, kind="ExternalInput").ap()
    wqT = nc.dram_tensor("wqT", [D, HD], bf16, kind="ExternalInput").ap()
    wkT = nc.dram_tensor("wkT", [D, HD], bf16, kind="ExternalInput").ap()
    wrkT = nc.dram_tensor("wrkT", [D, HD], bf16, kind="ExternalInput").ap()
    wvT = nc.dram_tensor("wvT", [D, HD], bf16, kind="ExternalInput").ap()
    wrvT = nc.dram_tensor("wrvT", [D, HD], bf16, kind="ExternalInput").ap()
    woT = nc.dram_tensor("woT", [HD, D], bf16, kind="ExternalInput").ap()
    bq_pc = nc.dram_tensor("bq_pc", [128, 4], f32, kind="ExternalInput").ap()
    bk_pc = nc.dram_tensor("bk_pc", [128, 4], f32, kind="ExternalInput").ap()
    brk_pc = nc.dram_tensor("brk_pc", [128, 4], f32, kind="ExternalInput").ap()
    bv_bc = nc.dram_tensor("bv_bc", [128, HD], f32, kind="ExternalInput").ap()
    brv_bc = nc.dram_tensor("brv_bc", [128, HD], f32, kind="ExternalInput").ap()
    maskb = nc.dram_tensor("maskb", [128, NM], f32, kind="ExternalInput").ap()
    yT = nc.dram_tensor("yT", [D, LQ], f32, kind="ExternalOutput").ap()
    scr1 = nc.dram_tensor("scr1", [8, 2048], bf16, kind="Internal").ap()
    scr2 = nc.dram_tensor("scr2", [8, 2048], bf16, kind="Internal").ap()

    with tile.TileContext(nc) as tc:
        from contextlib import ExitStack

        with ExitStack() as ctx:
            # Persistent SBUF tensors.
            persist = ctx.enter_context(tc.tile_pool(name="persist", bufs=1))
            qT_sb = persist.tile([128, 4 * LQ], bf16, tag="qT")
            kT_sb = persist.tile([128, 4 * LKP], bf16, tag="kT")
            rkT_sb = persist.tile([128, 4 * LKP], bf16, tag="rkT")
            v_sb = persist.tile([128, NM * 8 * 65], bf16, tag="v")
            rv_sb = persist.tile([128, NM * 8 * 65], bf16, tag="rv")
            xf_sb = persist.tile([128, 4 * LQ], bf16, tag="xf")
            maskb_sb = persist.tile([128, NM], f32, tag="maskb")
            bq_sb = persist.tile([128, 4], f32, tag="bq")
            bk_sb = persist.tile([128, 4], f32, tag="bk")
            brk_sb = persist.tile([128, 4], f32, tag="brk")
            bv_sb = persist.tile([128, HD], f32, tag="bv")
            brv_sb = persist.tile([128, HD], f32, tag="brv")

            nc.sync.dma_start(out=maskb_sb[:], in_=maskb)
            nc.sync.dma_start(out=bq_sb[:], in_=bq_pc)
            nc.sync.dma_start(out=bk_sb[:], in_=bk_pc)
            nc.sync.dma_start(out=brk_sb[:], in_=brk_pc)
            nc.sync.dma_start(out=bv_sb[:], in_=bv_bc)
            nc.sync.dma_start(out=brv_sb[:], in_=brv_bc)

            # v/rv: [128, m, hp, two, 65], each head [v|1] (ones column last
            # accumulates the softmax denominator in the PV matmul's row 64).
            v5 = v_sb[:].rearrange("p (m hp t c) -> p m hp t c", m=NM, hp=4, t=2, c=65)
            rv5 = rv_sb[:].rearrange("p (m hp t c) -> p m hp t c", m=NM, hp=4, t=2, c=65)
            for vv in (v5, rv5):
                nc.vector.memset(vv[:, :, :, :, 64:65], 1.0)

            # Input tiles. Tag reuse: xq slots -> xv; wq -> wrv; wk -> wv;
            # wrk slots (sized for wo) -> wo.
            inp = ctx.enter_context(tc.tile_pool(name="inp", bufs=8))
            psum = ctx.enter_context(tc.tile_pool(name="psum", bufs=2, space="PSUM"))
            ppool = ctx.enter_context(tc.tile_pool(name="ppool", bufs=34))
            xsp = ctx.enter_context(tc.tile_pool(name="xsp", bufs=8))
            sgp = ctx.enter_context(tc.tile_pool(name="sgp", bufs=2))
            bcp = ctx.enter_context(tc.tile_pool(name="bcp", bufs=6))
            tp = ctx.enter_context(tc.tile_pool(name="tp", bufs=4))
            ysb = ctx.enter_context(tc.tile_pool(name="ysb", bufs=2))

            # ---- input DMA issue (sync queue: q,k,rk in need-order) ----
            xq_ch, wq_ch = [], []
            for k in range(8):
                w = inp.tile([128, HD], bf16, tag="w1", name=f"wq{k}")
                nc.sync.dma_start(out=w[:], in_=wqT[128 * k : 128 * k + 128, :])
                wq_ch.append(w)
                t = inp.tile([128, LQ], bf16, tag="xbig", name=f"xq{k}")
                nc.sync.dma_start(out=t[:], in_=xqT[128 * k : 128 * k + 128, :])
                xq_ch.append(t)
            xk_ch, wk_ch = [], []
            for k in range(8):
                w = inp.tile([128, HD], bf16, tag="w2", name=f"wk{k}")
                nc.sync.dma_start(out=w[:], in_=wkT[128 * k : 128 * k + 128, :])
                wk_ch.append(w)
                t = inp.tile([128, LKP], bf16, tag="xk", name=f"xk{k}")
                nc.sync.dma_start(out=t[:], in_=xkT[128 * k : 128 * k + 128, :])
                xk_ch.append(t)
            xr_ch, wrk_ch = [], []
            for k in range(8):
                w = inp.tile([128, 1024], bf16, tag="w3", name=f"wrk{k}")
                nc.sync.dma_start(out=w[:, 0:HD], in_=wrkT[128 * k : 128 * k + 128, :])
                wrk_ch.append(w)
                t = inp.tile([128, LKP], bf16, tag="xr", name=f"xr{k}")
                nc.sync.dma_start(out=t[:], in_=xrT[128 * k : 128 * k + 128, :])
                xr_ch.append(t)
            # v inputs + wv/wrv/wo on the gpsimd queue (slot-reuse WAR waits
            # live on that queue, off the critical sync queue).
            xv_ch, wv_ch, wrv_ch = [], [], []
            for k in range(8):
                w = inp.tile([128, HD], bf16, tag="w2", name=f"wv{k}")
                nc.gpsimd.dma_start(out=w[:], in_=wvT[128 * k : 128 * k + 128, :])
                wv_ch.append(w)
                t = inp.tile([128, LQ], bf16, tag="xbig", name=f"xv{k}")
                nc.gpsimd.dma_start(out=t[:, 0:LKP], in_=xvT[128 * k : 128 * k + 128, :])
                xv_ch.append(t)
            for k in range(8):
                w = inp.tile([128, HD], bf16, tag="w1", name=f"wrv{k}")
                nc.gpsimd.dma_start(out=w[:], in_=wrvT[128 * k : 128 * k + 128, :])
                wrv_ch.append(w)
            wo_ch = []
            for dc in range(4):
                w = inp.tile([128, 1024], bf16, tag="w3", name=f"wo{dc}")
                nc.gpsimd.dma_start(out=w[:], in_=woT[128 * dc : 128 * dc + 128, :])
                wo_ch.append(w)

            p_tiles = {}

            def emit_scores(dc, lqh, br):
                kt = kT_sb if br == 0 else rkT_sb
                qsl = slice(1024 * dc + 512 * lqh, 1024 * dc + 512 * lqh + 512)
                for m in range(NM):
                    ksl = slice(LKP * dc + 128 * m, LKP * dc + 128 * m + 128)
                    s = psum.tile([128, 1024], f32, tag="spool", name="s")
                    nc.tensor.matmul(s[:, 0:512], kt[0:64, ksl], qT_sb[0:64, qsl])
                    nc.tensor.matmul(s[:, 512:1024], kt[64:128, ksl], qT_sb[64:128, qsl])
                    p = ppool.tile([128, 1024], bf16, tag="ppool", name="p")
                    nc.scalar.activation(
                        p[:], s[:], Exp, bias=maskb_sb[:, m : m + 1], scale=SCALE
                    )
                    p_tiles[(lqh, dc, m, br)] = p

            def proj_T(dc, xch, wch, b_sb, out_sb, LL):
                # Transposed projection chunk dc -> out_sb[:, LL*dc : LL*dc+LL]
                nsl = [slice(a, min(a + 512, LL)) for a in range(0, LL, 512)]
                for sl in nsl:
                    w = sl.stop - sl.start
                    ps = psum.tile([128, 512], f32, tag="bank", name="ps", bufs=4)
                    for k in range(8):
                        nc.tensor.matmul(
                            ps[:, 0:w],
                            wch[k][:, 128 * dc : 128 * dc + 128],
                            xch[k][:, sl],
                            start=(k == 0),
                            stop=(k == 7),
                        )
                    nc.vector.tensor_scalar(
                        out=out_sb[:, LL * dc + sl.start : LL * dc + sl.stop],
                        in0=ps[:, 0:w],
                        scalar1=b_sb[:, dc : dc + 1],
                        scalar2=None,
                        op0=Add,
                    )

            # ---- phase A: q/k/rk projections + lqh0 scores, dc-granular ----
            for dc in range(4):
                proj_T(dc, xq_ch, wq_ch, bq_sb, qT_sb, LQ)
                proj_T(dc, xk_ch, wk_ch, bk_sb, kT_sb, LKP)
                emit_scores(dc, 0, 0)
                proj_T(dc, xr_ch, wrk_ch, brk_sb, rkT_sb, LKP)
                emit_scores(dc, 0, 1)

            # ---- phase B: v/rv projections (natural orientation) ----
            bvv = bv_sb[:].rearrange("p (hp t c) -> p hp t c", hp=4, t=2, c=64)
            brvv = brv_sb[:].rearrange("p (hp t c) -> p hp t c", hp=4, t=2, c=64)
            for xch, wch, bview, out5 in ((xv_ch, wv_ch, bvv, v5), (xr_ch, wrv_ch, brvv, rv5)):
                for m in range(NM):
                    ps = psum.tile([128, 512], f32, tag="bank", name="psv", bufs=4)
                    for k in range(8):
                        nc.tensor.matmul(
                            ps[:, 0:512],
                            xch[k][:, 128 * m : 128 * m + 128],
                            wch[k][:],
                            start=(k == 0),
                            stop=(k == 7),
                        )
                    pv = ps[:, 0:512].rearrange("p (hp t c) -> p hp t c", hp=4, t=2, c=64)
                    nc.vector.tensor_tensor(
                        out=out5[:, m, :, :, 0:64], in0=pv[:], in1=bview[:], op=Add
                    )

            def emit_pv_chain(lqh, dc, use_spool=False):
                # PV accumulators: 4 regions [65,512] keyed (br, hs), either 4
                # one-bank slots or halves of 2 spool slots (phase D pipelining).
                xa = {}
                if use_spool:
                    for br in range(2):
                        t = psum.tile([128, 1024], f32, tag="spool", name=f"xas{br}")
                        xa[(br, 0)] = t[0:65, 0:512]
                        xa[(br, 1)] = t[0:65, 512:1024]
                else:
                    for br in range(2):
                        for hs in range(2):
                            t = psum.tile(
                                [128, 512], f32, tag="bank", name=f"xa{br}{hs}", bufs=4
                            )
                            xa[(br, hs)] = t[0:65, :]
                for br in range(2):
                    vv = v5 if br == 0 else rv5
                    for m in range(NM):
                        pt = p_tiles[(lqh, dc, m, br)]
                        for hs in range(2):
                            nc.tensor.matmul(
                                xa[(br, hs)],
                                vv[:, m, dc, hs, 0:65],
                                pt[:, 512 * hs : 512 * hs + 512],
                                start=(m == 0),
                                stop=(m == NM - 1),
                            )
                it = 2 * dc + lqh
                # Wide psum->SBUF bf16 copies (x rows + denominator row), then
                # den rows -> DRAM bounce -> [128,16] lanes -> recip -> bcast.
                xs = {}
                for j, (br, hs) in enumerate(((0, 0), (1, 0), (0, 1), (1, 1))):
                    t = xsp.tile([65, 512], bf16, tag="xs", name=f"xs{j}", bufs=8)
                    nc.vector.tensor_copy(out=t[:], in_=xa[(br, hs)])
                    xs[(br, hs)] = t
                    nc.sync.dma_start(
                        out=scr1[it, 512 * j : 512 * j + 512], in_=t[64:65, :]
                    )
                sgi = sgp.tile([128, 16], bf16, tag="sgi")
                nc.sync.dma_start(out=sgi[:], in_=scr1[it, :])
                sgo = sgp.tile([128, 16], bf16, tag="sgo")
                with nc.allow_low_precision(reason="bf16 1/denom, ~0.4% rel"):
                    nc.vector.reciprocal(sgo[:], sgi[:])
                nc.sync.dma_start(out=scr2[it, :], in_=sgo[:])
                bcs = []
                for j in range(4):
                    bc = bcp.tile([64, 512], bf16, tag="bcp", name=f"bc{j}")
                    nc.gpsimd.dma_start(
                        out=bc[:],
                        in_=scr2[it : it + 1, 512 * j : 512 * j + 512]
                        .partition_broadcast(64)[:, 0, :],
                    )
                    bcs.append(bc)
                xfsl = slice(1024 * dc + 512 * lqh, 1024 * dc + 512 * lqh + 512)
                t1 = tp.tile([64, 512], bf16, tag="tp", name="t1")
                t2 = tp.tile([64, 512], bf16, tag="tp", name="t2")
                nc.vector.tensor_tensor(
                    out=t1[:], in0=xs[(0, 0)][0:64, :], in1=bcs[0][:], op=Mult
                )
                nc.vector.tensor_tensor(
                    out=t2[:], in0=xs[(1, 0)][0:64, :], in1=bcs[1][:], op=Mult
                )
                nc.vector.tensor_tensor(
                    out=xf_sb[0:64, xfsl], in0=t1[:], in1=t2[:], op=Add
                )
                t3 = tp.tile([64, 512], bf16, tag="tp", name="t3")
                t4 = tp.tile([64, 512], bf16, tag="tp", name="t4")
                nc.vector.tensor_tensor(
                    out=t3[:], in0=xs[(0, 1)][0:64, :], in1=bcs[2][:], op=Mult
                )
                nc.vector.tensor_tensor(
                    out=t4[:], in0=xs[(1, 1)][0:64, :], in1=bcs[3][:], op=Mult
                )
                thi = tp.tile([64, 512], bf16, tag="thi", name="thi", bufs=2)
                nc.vector.tensor_tensor(out=thi[:], in0=t3[:], in1=t4[:], op=Add)
                nc.sync.dma_start(out=xf_sb[64:128, xfsl], in_=thi[:])

            # ---- phase C: lqh1 scores + PV(lqh0) ----
            for dc in range(4):
                emit_pv_chain(0, dc)
                emit_scores(dc, 1, 0)
                emit_scores(dc, 1, 1)

            def emit_y_out(reg, ot, lqh):
                # PSUM -> SBUF copy on the scalar engine (idle once exp is
                # done, which is before any outproj runs), then DMA out.
                y = ysb.tile([128, 512], f32, tag="ysb", name="y")
                nc.scalar.copy(y[:], reg)
                nc.sync.dma_start(
                    out=yT[128 * ot : 128 * ot + 128, 512 * lqh : 512 * lqh + 512],
                    in_=y[:],
                )

            def emit_outproj(lqh, wide):
                if not wide:
                    for pair in range(4):
                        yt2 = psum.tile([128, 1024], f32, tag="spool", name="yt2")
                        for half in range(2):
                            ot = 2 * pair + half
                            reg = yt2[:, 512 * half : 512 * half + 512]
                            for dc in range(4):
                                nc.tensor.matmul(
                                    reg,
                                    wo_ch[dc][:, 128 * ot : 128 * ot + 128],
                                    xf_sb[:, 1024 * dc + 512 * lqh : 1024 * dc + 512 * lqh + 512],
                                    start=(dc == 0),
                                    stop=(dc == 3),
                                )
                            emit_y_out(reg, ot, lqh)
                    return
                # Wide: 8 accumulators (2 prj + 2 spool slots, 2 halves each),
                # dc-outer so only the last dim-chunk waits on the last chain.
                regs = []
                for i in range(4):
                    w2 = psum.tile([128, 512], f32, tag="bank", name=f"ywb{i}", bufs=4)
                    regs.append(w2[:])
                for i in range(2):
                    w2 = psum.tile([128, 1024], f32, tag="spool", name=f"yws{i}")
                    regs.append(w2[:, 0:512])
                    regs.append(w2[:, 512:1024])
                for dc in range(4):
                    for ot in range(8):
                        nc.tensor.matmul(
                            regs[ot],
                            wo_ch[dc][:, 128 * ot : 128 * ot + 128],
                            xf_sb[:, 1024 * dc + 512 * lqh : 1024 * dc + 512 * lqh + 512],
                            start=(dc == 0),
                            stop=(dc == 3),
                        )
                for ot in range(8):
                    emit_y_out(regs[ot], ot, lqh)

            # ---- phase D: outproj(lqh0) then PV(lqh1) pipelined ----
            emit_outproj(0, wide=False)
            emit_pv_chain(1, 0, use_spool=False)
            emit_pv_chain(1, 1, use_spool=True)
            emit_pv_chain(1, 2, use_spool=False)
            emit_pv_chain(1, 3, use_spool=True)
            emit_outproj(1, wide=True)

    nc.compile()
    return nc


def _get_program(lkp=LKP):
    if lkp not in _CACHE:
        _CACHE[lkp] = _build_program(lkp)
    return _CACHE[lkp]


def _cast_bf16(arr):
    import ml_dtypes

    return np.ascontiguousarray(arr.astype(ml_dtypes.bfloat16))


def _shard_inputs(inputs, lkp=LKP):
    q = np.ascontiguousarray(inputs["query"], dtype=np.float32)
    k = np.ascontiguousarray(inputs["key"], dtype=np.float32)
    v = np.ascontiguousarray(inputs["value"], dtype=np.float32)
    wr = np.ascontiguousarray(inputs["weak_rela"], dtype=np.float32)
    mask = np.asarray(inputs["mask"])

    in_maps = []
    for c in range(N_CORES):
        b, hh = divmod(c, 2)
        hsl = slice(HD * hh, HD * hh + HD)
        idx = np.nonzero(mask[b, 0])[0]
        nv = len(idx)
        assert nv <= lkp
        pidx = np.concatenate([idx, np.zeros(lkp - nv, dtype=idx.dtype)])
        bias = np.full(lkp, -1.0e9, np.float32)
        bias[:nv] = 0.0
        mb = np.ascontiguousarray(bias.reshape(lkp // 128, 128).T)
        kc, vc, wrc = k[b][pidx], v[b][pidx], wr[b][pidx]
        m = {
            "xqT": _cast_bf16(q[b].T),
            "xkT": _cast_bf16(kc.T),
            "xrT": _cast_bf16(wrc.T),
            "xvT": _cast_bf16(vc.T),
            "wqT": _cast_bf16(np.asarray(inputs["Wq"])[hsl, :].T),
            "wkT": _cast_bf16(np.asarray(inputs["Wk"])[hsl, :].T),
            "wrkT": _cast_bf16(np.asarray(inputs["Wrk"])[hsl, :].T),
            "wvT": _cast_bf16(np.asarray(inputs["Wv"])[hsl, :].T),
            "wrvT": _cast_bf16(np.asarray(inputs["Wrv"])[hsl, :].T),
            "woT": _cast_bf16(np.asarray(inputs["Wo"])[:, hsl].T),
            "bq_pc": np.asarray(inputs["bq"][hsl]).reshape(4, 128).T.astype(np.float32),
            "bk_pc": np.asarray(inputs["bk"][hsl]).reshape(4, 128).T.astype(np.float32),
            "brk_pc": np.asarray(inputs["brk"][hsl])
            .reshape(4, 128)
            .T.astype(np.float32),
            "bv_bc": np.broadcast_to(inputs["bv"][hsl], (128, HD)).astype(np.float32),
            "brv_bc": np.broadcast_to(inputs["brv"][hsl], (128, HD)).astype(np.float32),
            "maskb": mb,
        }
        in_maps.append({k2: np.ascontiguousarray(v2) for k2, v2 in m.items()})
    return in_maps


def run_on_hw(inputs, trace=False, **kw):
    from concourse.bass_utils import run_bass_kernel_spmd

    mask = np.asarray(inputs["mask"])
    max_valid = max(int(mask[b, 0].sum()) for b in range(B))
    lkp = max(LKP, ((max_valid + 127) // 128) * 128)
    nc = _get_program(lkp)
    in_maps = _shard_inputs(inputs, lkp)
    res = run_bass_kernel_spmd(
        nc, in_maps, core_ids=list(range(N_CORES)), trace=trace, **kw
    )
    bo = np.asarray(inputs["bo"], dtype=np.float32)
    outs = []
    for b in range(B):
        yt = res.results[2 * b]["yT"] + res.results[2 * b + 1]["yT"]
        outs.append(yt.T + bo)
    out = np.stack(outs).astype(np.float32)
    return out, res


def kernel(**inputs):
    out, _ = run_on_hw(inputs)
    return out


# revision 19
# speedup vs baseline: 1.5770x; 1.1023x over previous
"""Fused multi-head cross-attention with relation branch, sharded over 8 NeuronCores.

Sharding: data-parallel over batch (4) x tensor-parallel over head halves (2).
Core c handles batch c//2, heads [8*(c%2), 8*(c%2)+8). Each core computes its
partial output projection; the host sums the two partials per batch and adds bo.

v2 schedule: dc-granular interleave so the scalar (exp) engine starts ~16us in
instead of ~57us, normalize chains read PV psum directly (no staging copies),
denominator rows DMA straight from PSUM to the DRAM reshape bounce, odd heads
carry their softmax-denominator ones-column FIRST so their PV accumulator sits
at psum partitions 63..127 and the normalized add writes xf rows 64..127 with
no partition-shift DMA, and the output projection DMAs straight from PSUM.

Emission order (tensor stream):
  for dc: q-proj(dc), k-proj(dc), rk-proj(dc), scores(dc, lqh=0, vis|rel)
  v-proj, rv-proj
  for dc: PV(lqh0, dc)+chain, scores(dc, lqh=1, vis|rel)
  PV(lqh1, 0)+chain, outproj(lqh0), PV(lqh1, 1..3)+chain, outproj(lqh1, wide)

PSUM: one pool, tags "prj" (2 slots x 2 banks: projections, PV accumulators,
wide outproj) + "spool" (2 slots x 2 banks: score tiles, outproj lqh0) = 8 banks.
"""

import math

import numpy as np

B, LQ, LK, D, H = 4, 1024, 1024, 1024, 16
DK = D // H
SCALE = 1.0 / math.sqrt(DK)
N_CORES = 8
HD = D // 2  # local dims per core (8 heads * 64)
# Keys are compacted host-side: only unmasked keys are shipped (padded to LKP
# with dummy rows whose mask bias is -1e9, so exp()=0 -> exact same math).
LKP = 640
NM = LKP // 128  # lk chunks

_CACHE = {}


def _build_program(lkp=LKP):
    import concourse.bacc as bacc
    import concourse.mybir as mybir
    import concourse.tile as tile

    LKP = lkp
    NM = LKP // 128

    f32 = mybir.dt.float32
    bf16 = mybir.dt.bfloat16
    Exp = mybir.ActivationFunctionType.Exp
    Add = mybir.AluOpType.add
    Mult = mybir.AluOpType.mult

    nc = bacc.Bacc(
        "TRN2",
        target_bir_lowering=False,
        debug=False,
        enable_asserts=False,
        num_devices=N_CORES,
    )

    # DRAM I/O (per-core shapes; host shards/pre-transposes/casts).
    xqT = nc.dram_tensor("xqT", [D, LQ], bf16, kind="ExternalInput").ap()
    xkT = nc.dram_tensor("xkT", [D, LKP], bf16, kind="ExternalInput").ap()
    xrT = nc.dram_tensor("xrT", [D, LKP], bf16, kind="ExternalInput").ap()
    xvT = nc.dram_tensor("xvT", [D, LKP], bf16, kind="ExternalInput").ap()
    wqT = nc.dram_tensor("wqT", [D, HD], bf16, kind="ExternalInput").ap()
    wkT = nc.dram_tensor("wkT", [D, HD], bf16, kind="ExternalInput").ap()
    wrkT = nc.dram_tensor("wrkT", [D, HD], bf16, kind="ExternalInput").ap()
    wvT = nc.dram_tensor("wvT", [D, HD], bf16, kind="ExternalInput").ap()
    wrvT = nc.dram_tensor("wrvT", [D, HD], bf16, kind="ExternalInput").ap()
    woT = nc.dram_tensor("woT", [HD, D], bf16, kind="ExternalInput").ap()
    bq_pc = nc.dram_tensor("bq_pc", [128, 4], f32, kind="ExternalInput").ap()
    bk_pc = nc.dram_tensor("bk_pc", [128, 4], f32, kind="ExternalInput").ap()
    brk_pc = nc.dram_tensor("brk_pc", [128, 4], f32, kind="ExternalInput").ap()
    bv_bc = nc.dram_tensor("bv_bc", [128, HD], f32, kind="ExternalInput").ap()
    brv_bc = nc.dram_tensor("brv_bc", [128, HD], f32, kind="ExternalInput").ap()
    maskb = nc.dram_tensor("maskb", [128, NM], f32, kind="ExternalInput").ap()
    yT = nc.dram_tensor("yT", [D, LQ], f32, kind="ExternalOutput").ap()
    scr1 = nc.dram_tensor("scr1", [8, 2048], bf16, kind="Internal").ap()
    scr2 = nc.dram_tensor("scr2", [8, 2048], bf16, kind="Internal").ap()

    with tile.TileContext(nc) as tc:
        from contextlib import ExitStack

        with ExitStack() as ctx:
            # Persistent SBUF tensors.
            persist = ctx.enter_context(tc.tile_pool(name="persist", bufs=1))
            qT_sb = persist.tile([128, 4 * LQ], bf16, tag="qT")
            kT_sb = persist.tile([128, 4 * LKP], bf16, tag="kT")
            rkT_sb = persist.tile([128, 4 * LKP], bf16, tag="rkT")
            v_sb = persist.tile([128, NM * 8 * 65], bf16, tag="v")
            rv_sb = persist.tile([128, NM * 8 * 65], bf16, tag="rv")
            xf_sb = persist.tile([128, 4 * LQ], bf16, tag="xf")
            maskb_sb = persist.tile([128, NM], f32, tag="maskb")
            bq_sb = persist.tile([128, 4], f32, tag="bq")
            bk_sb = persist.tile([128, 4], f32, tag="bk")
            brk_sb = persist.tile([128, 4], f32, tag="brk")
            bv_sb = persist.tile([128, HD], f32, tag="bv")
            brv_sb = persist.tile([128, HD], f32, tag="brv")

            nc.sync.dma_start(out=maskb_sb[:], in_=maskb)
            nc.sync.dma_start(out=bq_sb[:], in_=bq_pc)
            nc.sync.dma_start(out=bk_sb[:], in_=bk_pc)
            nc.sync.dma_start(out=brk_sb[:], in_=brk_pc)
            nc.sync.dma_start(out=bv_sb[:], in_=bv_bc)
            nc.sync.dma_start(out=brv_sb[:], in_=brv_bc)

            # v/rv: [128, m, hp, two, 65], each head [v|1] (ones column last
            # accumulates the softmax denominator in the PV matmul's row 64).
            v5 = v_sb[:].rearrange("p (m hp t c) -> p m hp t c", m=NM, hp=4, t=2, c=65)
            rv5 = rv_sb[:].rearrange("p (m hp t c) -> p m hp t c", m=NM, hp=4, t=2, c=65)
            for vv in (v5, rv5):
                nc.vector.memset(vv[:, :, :, :, 64:65], 1.0)

            # Input tiles. Tag reuse: xq slots -> xv; wq -> wrv; wk -> wv;
            # wrk slots (sized for wo) -> wo.
            inp = ctx.enter_context(tc.tile_pool(name="inp", bufs=8))
            psum = ctx.enter_context(tc.tile_pool(name="psum", bufs=2, space="PSUM"))
            ppool = ctx.enter_context(tc.tile_pool(name="ppool", bufs=34))
            xsp = ctx.enter_context(tc.tile_pool(name="xsp", bufs=8))
            sgp = ctx.enter_context(tc.tile_pool(name="sgp", bufs=2))
            bcp = ctx.enter_context(tc.tile_pool(name="bcp", bufs=2))
            tp = ctx.enter_context(tc.tile_pool(name="tp", bufs=4))
            ysb = ctx.enter_context(tc.tile_pool(name="ysb", bufs=2))

            # ---- input DMA issue (sync queue: q,k,rk in need-order) ----
            xq_ch, wq_ch = [], []
            for k in range(8):
                w = inp.tile([128, HD], bf16, tag="w1", name=f"wq{k}")
                nc.sync.dma_start(out=w[:], in_=wqT[128 * k : 128 * k + 128, :])
                wq_ch.append(w)
                t = inp.tile([128, LQ], bf16, tag="xbig", name=f"xq{k}")
                nc.sync.dma_start(out=t[:], in_=xqT[128 * k : 128 * k + 128, :])
                xq_ch.append(t)
            xk_ch, wk_ch = [], []
            for k in range(8):
                w = inp.tile([128, HD], bf16, tag="w2", name=f"wk{k}")
                nc.sync.dma_start(out=w[:], in_=wkT[128 * k : 128 * k + 128, :])
                wk_ch.append(w)
                t = inp.tile([128, LKP], bf16, tag="xk", name=f"xk{k}")
                nc.sync.dma_start(out=t[:], in_=xkT[128 * k : 128 * k + 128, :])
                xk_ch.append(t)
            xr_ch, wrk_ch = [], []
            for k in range(8):
                w = inp.tile([128, 1024], bf16, tag="w3", name=f"wrk{k}")
                nc.sync.dma_start(out=w[:, 0:HD], in_=wrkT[128 * k : 128 * k + 128, :])
                wrk_ch.append(w)
                t = inp.tile([128, LKP], bf16, tag="xr", name=f"xr{k}")
                nc.sync.dma_start(out=t[:], in_=xrT[128 * k : 128 * k + 128, :])
                xr_ch.append(t)
            # v inputs + wv/wrv/wo on the gpsimd queue (slot-reuse WAR waits
            # live on that queue, off the critical sync queue).
            xv_ch, wv_ch, wrv_ch = [], [], []
            for k in range(8):
                w = inp.tile([128, HD], bf16, tag="w2", name=f"wv{k}")
                nc.gpsimd.dma_start(out=w[:], in_=wvT[128 * k : 128 * k + 128, :])
                wv_ch.append(w)
                t = inp.tile([128, LQ], bf16, tag="xbig", name=f"xv{k}")
                nc.gpsimd.dma_start(out=t[:, 0:LKP], in_=xvT[128 * k : 128 * k + 128, :])
                xv_ch.append(t)
            for k in range(8):
                w = inp.tile([128, HD], bf16, tag="w1", name=f"wrv{k}")
                nc.gpsimd.dma_start(out=w[:], in_=wrvT[128 * k : 128 * k + 128, :])
                wrv_ch.append(w)
            wo_ch = []
            for dc in range(4):
                w = inp.tile([128, 1024], bf16, tag="w3", name=f"wo{dc}")
                nc.gpsimd.dma_start(out=w[:], in_=woT[128 * dc : 128 * dc + 128, :])
                wo_ch.append(w)

            p_tiles = {}

            def emit_scores(dc, lqh, br):
                kt = kT_sb if br == 0 else rkT_sb
                qsl = slice(1024 * dc + 512 * lqh, 1024 * dc + 512 * lqh + 512)
                for m in range(NM):
                    ksl = slice(LKP * dc + 128 * m, LKP * dc + 128 * m + 128)
                    s = psum.tile([128, 1024], f32, tag="spool", name="s")
                    nc.tensor.matmul(s[:, 0:512], kt[0:64, ksl], qT_sb[0:64, qsl])
                    nc.tensor.matmul(s[:, 512:1024], kt[64:128, ksl], qT_sb[64:128, qsl])
                    p = ppool.tile([128, 1024], bf16, tag="ppool", name="p")
                    nc.scalar.activation(
                        p[:], s[:], Exp, bias=maskb_sb[:, m : m + 1], scale=SCALE
                    )
                    p_tiles[(lqh, dc, m, br)] = p

            def proj_T(dc, xch, wch, b_sb, out_sb, LL):
                # Transposed projection chunk dc -> out_sb[:, LL*dc : LL*dc+LL].
                # Chunk-outer so each k-chunk's matmuls issue as its DMA lands.
                nsl = [slice(a, min(a + 512, LL)) for a in range(0, LL, 512)]
                pss = [
                    psum.tile([128, 512], f32, tag="bank", name=f"ps{i}", bufs=4)
                    for i in range(len(nsl))
                ]
                for k in range(8):
                    for i, sl in enumerate(nsl):
                        nc.tensor.matmul(
                            pss[i][:, 0 : sl.stop - sl.start],
                            wch[k][:, 128 * dc : 128 * dc + 128],
                            xch[k][:, sl],
                            start=(k == 0),
                            stop=(k == 7),
                        )
                for i, sl in enumerate(nsl):
                    nc.vector.tensor_scalar(
                        out=out_sb[:, LL * dc + sl.start : LL * dc + sl.stop],
                        in0=pss[i][:, 0 : sl.stop - sl.start],
                        scalar1=b_sb[:, dc : dc + 1],
                        scalar2=None,
                        op0=Add,
                    )

            # ---- phase A: q/k/rk projections + lqh0 scores, dc-granular ----
            for dc in range(4):
                proj_T(dc, xq_ch, wq_ch, bq_sb, qT_sb, LQ)
                proj_T(dc, xk_ch, wk_ch, bk_sb, kT_sb, LKP)
                emit_scores(dc, 0, 0)
                proj_T(dc, xr_ch, wrk_ch, brk_sb, rkT_sb, LKP)
                emit_scores(dc, 0, 1)

            # ---- phase B: v/rv projections (natural orientation) ----
            bvv = bv_sb[:].rearrange("p (hp t c) -> p hp t c", hp=4, t=2, c=64)
            brvv = brv_sb[:].rearrange("p (hp t c) -> p hp t c", hp=4, t=2, c=64)
            for xch, wch, bview, out5 in ((xv_ch, wv_ch, bvv, v5), (xr_ch, wrv_ch, brvv, rv5)):
                for m in range(NM):
                    ps = psum.tile([128, 512], f32, tag="bank", name="psv", bufs=4)
                    for k in range(8):
                        nc.tensor.matmul(
                            ps[:, 0:512],
                            xch[k][:, 128 * m : 128 * m + 128],
                            wch[k][:],
                            start=(k == 0),
                            stop=(k == 7),
                        )
                    pv = ps[:, 0:512].rearrange("p (hp t c) -> p hp t c", hp=4, t=2, c=64)
                    nc.vector.tensor_tensor(
                        out=out5[:, m, :, :, 0:64], in0=pv[:], in1=bview[:], op=Add
                    )

            def emit_pv_chain(lqh, dc, use_spool=False):
                # PV accumulators: 4 regions [65,512] keyed (br, hs), either 4
                # one-bank slots or halves of 2 spool slots (phase D pipelining).
                xa = {}
                if use_spool:
                    for br in range(2):
                        t = psum.tile([128, 1024], f32, tag="spool", name=f"xas{br}")
                        xa[(br, 0)] = t[0:65, 0:512]
                        xa[(br, 1)] = t[0:65, 512:1024]
                else:
                    for br in range(2):
                        for hs in range(2):
                            t = psum.tile(
                                [128, 512], f32, tag="bank", name=f"xa{br}{hs}", bufs=4
                            )
                            xa[(br, hs)] = t[0:65, :]
                for br in range(2):
                    vv = v5 if br == 0 else rv5
                    for m in range(NM):
                        pt = p_tiles[(lqh, dc, m, br)]
                        for hs in range(2):
                            nc.tensor.matmul(
                                xa[(br, hs)],
                                vv[:, m, dc, hs, 0:65],
                                pt[:, 512 * hs : 512 * hs + 512],
                                start=(m == 0),
                                stop=(m == NM - 1),
                            )
                it = 2 * dc + lqh
                # Wide psum->SBUF bf16 copies packed into one tile (x rows +
                # denominator row), one den-row DMA to the DRAM bounce, one
                # [128,16]-lane reciprocal, one wide partition-broadcast back.
                # For lqh1 chains the scalar engine (idle, exp done) takes two
                # of the four copies to halve the copy-stage latency.
                xs_all = xsp.tile([65, 2048], bf16, tag="xs", name="xs_all", bufs=2)
                xs = {}
                for j, (br, hs) in enumerate(((0, 0), (1, 0), (0, 1), (1, 1))):
                    sl_ = xs_all[:, 512 * j : 512 * j + 512]
                    if lqh == 1 and j >= 2:
                        nc.scalar.copy(sl_, xa[(br, hs)])
                    else:
                        nc.vector.tensor_copy(out=sl_, in_=xa[(br, hs)])
                    xs[(br, hs)] = sl_
                nc.sync.dma_start(out=scr1[it, :], in_=xs_all[64:65, :])
                sgi = sgp.tile([128, 16], bf16, tag="sgi")
                nc.sync.dma_start(out=sgi[:], in_=scr1[it, :])
                sgo = sgp.tile([128, 16], bf16, tag="sgo")
                with nc.allow_low_precision(reason="bf16 1/denom, ~0.4% rel"):
                    nc.vector.reciprocal(sgo[:], sgi[:])
                nc.sync.dma_start(out=scr2[it, :], in_=sgo[:])
                bca = bcp.tile([64, 2048], bf16, tag="bcp", name="bca")
                nc.gpsimd.dma_start(
                    out=bca[:],
                    in_=scr2[it : it + 1, :].partition_broadcast(64)[:, 0, :],
                )
                bcs = [bca[:, 512 * j : 512 * j + 512] for j in range(4)]
                xfsl = slice(1024 * dc + 512 * lqh, 1024 * dc + 512 * lqh + 512)
                t1 = tp.tile([64, 512], bf16, tag="tp", name="t1")
                t2 = tp.tile([64, 512], bf16, tag="tp", name="t2")
                nc.vector.tensor_tensor(
                    out=t1[:], in0=xs[(0, 0)][0:64, :], in1=bcs[0][:], op=Mult
                )
                nc.vector.tensor_tensor(
                    out=t2[:], in0=xs[(1, 0)][0:64, :], in1=bcs[1][:], op=Mult
                )
                nc.vector.tensor_tensor(
                    out=xf_sb[0:64, xfsl], in0=t1[:], in1=t2[:], op=Add
                )
                t3 = tp.tile([64, 512], bf16, tag="tp", name="t3")
                t4 = tp.tile([64, 512], bf16, tag="tp", name="t4")
                nc.vector.tensor_tensor(
                    out=t3[:], in0=xs[(0, 1)][0:64, :], in1=bcs[2][:], op=Mult
                )
                nc.vector.tensor_tensor(
                    out=t4[:], in0=xs[(1, 1)][0:64, :], in1=bcs[3][:], op=Mult
                )
                thi = tp.tile([64, 512], bf16, tag="thi", name="thi", bufs=2)
                nc.vector.tensor_tensor(out=thi[:], in0=t3[:], in1=t4[:], op=Add)
                nc.gpsimd.dma_start(out=xf_sb[64:128, xfsl], in_=thi[:])

            # ---- phase C: lqh1 scores + PV(lqh0) ----
            for dc in range(4):
                emit_pv_chain(0, dc)
                emit_scores(dc, 1, 0)
                emit_scores(dc, 1, 1)

            def emit_y_out(reg, ot, lqh):
                # PSUM -> SBUF copy on the scalar engine (idle once exp is
                # done, which is before any outproj runs), then DMA out.
                y = ysb.tile([128, 512], f32, tag="ysb", name="y")
                nc.scalar.copy(y[:], reg)
                nc.sync.dma_start(
                    out=yT[128 * ot : 128 * ot + 128, 512 * lqh : 512 * lqh + 512],
                    in_=y[:],
                )

            def emit_outproj(lqh, wide):
                if not wide:
                    for pair in range(4):
                        yt2 = psum.tile([128, 1024], f32, tag="spool", name="yt2")
                        for half in range(2):
                            ot = 2 * pair + half
                            reg = yt2[:, 512 * half : 512 * half + 512]
                            for dc in range(4):
                                nc.tensor.matmul(
                                    reg,
                                    wo_ch[dc][:, 128 * ot : 128 * ot + 128],
                                    xf_sb[:, 1024 * dc + 512 * lqh : 1024 * dc + 512 * lqh + 512],
                                    start=(dc == 0),
                                    stop=(dc == 3),
                                )
                            emit_y_out(reg, ot, lqh)
                    return
                # Wide: 8 accumulators (2 prj + 2 spool slots, 2 halves each),
                # dc-outer so only the last dim-chunk waits on the last chain.
                regs = []
                for i in range(4):
                    w2 = psum.tile([128, 512], f32, tag="bank", name=f"ywb{i}", bufs=4)
                    regs.append(w2[:])
                for i in range(2):
                    w2 = psum.tile([128, 1024], f32, tag="spool", name=f"yws{i}")
                    regs.append(w2[:, 0:512])
                    regs.append(w2[:, 512:1024])
                for dc in range(4):
                    for ot in range(8):
                        nc.tensor.matmul(
                            regs[ot],
                            wo_ch[dc][:, 128 * ot : 128 * ot + 128],
                            xf_sb[:, 1024 * dc + 512 * lqh : 1024 * dc + 512 * lqh + 512],
                            start=(dc == 0),
                            stop=(dc == 3),
                        )
                for ot in range(8):
                    emit_y_out(regs[ot], ot, lqh)

            # ---- phase D: outproj(lqh0) then PV(lqh1) pipelined ----
            emit_outproj(0, wide=False)
            emit_pv_chain(1, 0, use_spool=False)
            emit_pv_chain(1, 1, use_spool=True)
            emit_pv_chain(1, 2, use_spool=False)
            emit_pv_chain(1, 3, use_spool=True)
            emit_outproj(1, wide=True)

    nc.compile()
    return nc


def _get_program(lkp=LKP):
    if lkp not in _CACHE:
        _CACHE[lkp] = _build_program(lkp)
    return _CACHE[lkp]


def _cast_bf16(arr):
    import ml_dtypes

    return np.ascontiguousarray(arr.astype(ml_dtypes.bfloat16))


def _shard_inputs(inputs, lkp=LKP):
    q = np.ascontiguousarray(inputs["query"], dtype=np.float32)
    k = np.ascontiguousarray(inputs["key"], dtype=np.float32)
    v = np.ascontiguousarray(inputs["value"], dtype=np.float32)
    wr = np.ascontiguousarray(inputs["weak_rela"], dtype=np.float32)
    mask = np.asarray(inputs["mask"])

    in_maps = []
    for c in range(N_CORES):
        b, hh = divmod(c, 2)
        hsl = slice(HD * hh, HD * hh + HD)
        idx = np.nonzero(mask[b, 0])[0]
        nv = len(idx)
        assert nv <= lkp
        pidx = np.concatenate([idx, np.zeros(lkp - nv, dtype=idx.dtype)])
        bias = np.full(lkp, -1.0e9, np.float32)
        bias[:nv] = 0.0
        mb = np.ascontiguousarray(bias.reshape(lkp // 128, 128).T)
        kc, vc, wrc = k[b][pidx], v[b][pidx], wr[b][pidx]
        m = {
            "xqT": _cast_bf16(q[b].T),
            "xkT": _cast_bf16(kc.T),
            "xrT": _cast_bf16(wrc.T),
            "xvT": _cast_bf16(vc.T),
            "wqT": _cast_bf16(np.asarray(inputs["Wq"])[hsl, :].T),
            "wkT": _cast_bf16(np.asarray(inputs["Wk"])[hsl, :].T),
            "wrkT": _cast_bf16(np.asarray(inputs["Wrk"])[hsl, :].T),
            "wvT": _cast_bf16(np.asarray(inputs["Wv"])[hsl, :].T),
            "wrvT": _cast_bf16(np.asarray(inputs["Wrv"])[hsl, :].T),
            "woT": _cast_bf16(np.asarray(inputs["Wo"])[:, hsl].T),
            "bq_pc": np.asarray(inputs["bq"][hsl]).reshape(4, 128).T.astype(np.float32),
            "bk_pc": np.asarray(inputs["bk"][hsl]).reshape(4, 128).T.astype(np.float32),
            "brk_pc": np.asarray(inputs["brk"][hsl])
            .reshape(4, 128)
            .T.astype(np.float32),
            "bv_bc": np.broadcast_to(inputs["bv"][hsl], (128, HD)).astype(np.float32),
            "brv_bc": np.broadcast_to(inputs["brv"][hsl], (128, HD)).astype(np.float32),
            "maskb": mb,
        }
        in_maps.append({k2: np.ascontiguousarray(v2) for k2, v2 in m.items()})
    return in_maps


def run_on_hw(inputs, trace=False, **kw):
    from concourse.bass_utils import run_bass_kernel_spmd

    mask = np.asarray(inputs["mask"])
    max_valid = max(int(mask[b, 0].sum()) for b in range(B))
    lkp = max(LKP, ((max_valid + 127) // 128) * 128)
    nc = _get_program(lkp)
    in_maps = _shard_inputs(inputs, lkp)
    res = run_bass_kernel_spmd(
        nc, in_maps, core_ids=list(range(N_CORES)), trace=trace, **kw
    )
    bo = np.asarray(inputs["bo"], dtype=np.float32)
    outs = []
    for b in range(B):
        yt = res.results[2 * b]["yT"] + res.results[2 * b + 1]["yT"]
        outs.append(yt.T + bo)
    out = np.stack(outs).astype(np.float32)
    return out, res


def kernel(**inputs):
    out, _ = run_on_hw(inputs)
    return out


# revision 23
# speedup vs baseline: 1.7338x; 1.0994x over previous
"""Fused multi-head cross-attention with relation branch, sharded over 8 NeuronCores.

Sharding: data-parallel over batch (4) x tensor-parallel over head halves (2).
Core c handles batch c//2, heads [8*(c%2), 8*(c%2)+8). Each core computes its
partial output projection; the host sums the two partials per batch and adds bo.

v2 schedule: dc-granular interleave so the scalar (exp) engine starts ~16us in
instead of ~57us, normalize chains read PV psum directly (no staging copies),
denominator rows DMA straight from PSUM to the DRAM reshape bounce, odd heads
carry their softmax-denominator ones-column FIRST so their PV accumulator sits
at psum partitions 63..127 and the normalized add writes xf rows 64..127 with
no partition-shift DMA, and the output projection DMAs straight from PSUM.

Emission order (tensor stream):
  for dc: q-proj(dc), k-proj(dc), rk-proj(dc), scores(dc, lqh=0, vis|rel)
  v-proj, rv-proj
  for dc: PV(lqh0, dc)+chain, scores(dc, lqh=1, vis|rel)
  PV(lqh1, 0)+chain, outproj(lqh0), PV(lqh1, 1..3)+chain, outproj(lqh1, wide)

PSUM: one pool, tags "prj" (2 slots x 2 banks: projections, PV accumulators,
wide outproj) + "spool" (2 slots x 2 banks: score tiles, outproj lqh0) = 8 banks.
"""

import math

import numpy as np

B, LQ, LK, D, H = 4, 1024, 1024, 1024, 16
DK = D // H
SCALE = 1.0 / math.sqrt(DK)
N_CORES = 8
HD = D // 2  # local dims per core (8 heads * 64)
# Keys are compacted host-side: only unmasked keys are shipped (padded to LKP
# with dummy rows whose mask bias is -1e9, so exp()=0 -> exact same math).
LKP = 640
NM = LKP // 128  # lk chunks

_CACHE = {}


def _build_program(lkp=LKP):
    import concourse.bacc as bacc
    import concourse.mybir as mybir
    import concourse.tile as tile

    LKP = lkp
    NM = LKP // 128

    f32 = mybir.dt.float32
    bf16 = mybir.dt.bfloat16
    Exp = mybir.ActivationFunctionType.Exp
    Add = mybir.AluOpType.add
    Mult = mybir.AluOpType.mult

    nc = bacc.Bacc(
        "TRN2",
        target_bir_lowering=False,
        debug=False,
        enable_asserts=False,
        num_devices=N_CORES,
    )

    # DRAM I/O (per-core shapes; host shards/pre-transposes/casts).
    xqT = nc.dram_tensor("xqT", [D, LQ], bf16, kind="ExternalInput").ap()
    xkT = nc.dram_tensor("xkT", [D, LKP], bf16, kind="ExternalInput").ap()
    xrT = nc.dram_tensor("xrT", [D, LKP], bf16, kind="ExternalInput").ap()
    xvT = nc.dram_tensor("xvT", [D, LKP], bf16, kind="ExternalInput").ap()
    wqT = nc.dram_tensor("wqT", [D, HD], bf16, kind="ExternalInput").ap()
    wkT = nc.dram_tensor("wkT", [D, HD], bf16, kind="ExternalInput").ap()
    wrkT = nc.dram_tensor("wrkT", [D, HD], bf16, kind="ExternalInput").ap()
    wvT = nc.dram_tensor("wvT", [D, HD], bf16, kind="ExternalInput").ap()
    wrvT = nc.dram_tensor("wrvT", [D, HD], bf16, kind="ExternalInput").ap()
    woT = nc.dram_tensor("woT", [HD, D], bf16, kind="ExternalInput").ap()
    bq_pc = nc.dram_tensor("bq_pc", [128, 4], f32, kind="ExternalInput").ap()
    bk_pc = nc.dram_tensor("bk_pc", [128, 4], f32, kind="ExternalInput").ap()
    brk_pc = nc.dram_tensor("brk_pc", [128, 4], f32, kind="ExternalInput").ap()
    bv_bc = nc.dram_tensor("bv_bc", [128, HD], f32, kind="ExternalInput").ap()
    brv_bc = nc.dram_tensor("brv_bc", [128, HD], f32, kind="ExternalInput").ap()
    maskb = nc.dram_tensor("maskb", [128, NM], f32, kind="ExternalInput").ap()
    yT = nc.dram_tensor("yT", [D, LQ], bf16, kind="ExternalOutput").ap()
    scr1 = nc.dram_tensor("scr1", [8, 2048], bf16, kind="Internal").ap()
    scr2 = nc.dram_tensor("scr2", [8, 2048], bf16, kind="Internal").ap()

    with tile.TileContext(nc) as tc:
        from contextlib import ExitStack

        with ExitStack() as ctx:
            # Persistent SBUF tensors.
            persist = ctx.enter_context(tc.tile_pool(name="persist", bufs=1))
            qT_sb = persist.tile([128, 4 * LQ], bf16, tag="qT")
            kT_sb = persist.tile([128, 4 * LKP], bf16, tag="kT")
            rkT_sb = persist.tile([128, 4 * LKP], bf16, tag="rkT")
            v_sb = persist.tile([128, NM * 8 * 65], bf16, tag="v")
            rv_sb = persist.tile([128, NM * 8 * 65], bf16, tag="rv")
            xf_sb = persist.tile([128, 4 * LQ], bf16, tag="xf")
            maskb_sb = persist.tile([128, NM], f32, tag="maskb")
            bq_sb = persist.tile([128, 4], f32, tag="bq")
            bk_sb = persist.tile([128, 4], f32, tag="bk")
            brk_sb = persist.tile([128, 4], f32, tag="brk")
            bv_sb = persist.tile([128, HD], f32, tag="bv")
            brv_sb = persist.tile([128, HD], f32, tag="brv")

            nc.sync.dma_start(out=maskb_sb[:], in_=maskb)
            nc.sync.dma_start(out=bq_sb[:], in_=bq_pc)
            nc.sync.dma_start(out=bk_sb[:], in_=bk_pc)
            nc.sync.dma_start(out=brk_sb[:], in_=brk_pc)
            nc.sync.dma_start(out=bv_sb[:], in_=bv_bc)
            nc.sync.dma_start(out=brv_sb[:], in_=brv_bc)

            # v/rv: [128, m, hp, two, 65], each head [v|1] (ones column last
            # accumulates the softmax denominator in the PV matmul's row 64).
            v5 = v_sb[:].rearrange("p (m hp t c) -> p m hp t c", m=NM, hp=4, t=2, c=65)
            rv5 = rv_sb[:].rearrange("p (m hp t c) -> p m hp t c", m=NM, hp=4, t=2, c=65)
            for vv in (v5, rv5):
                nc.vector.memset(vv[:, :, :, :, 64:65], 1.0)

            # Input tiles. Tag reuse: xq slots -> xv; wq -> wrv; wk -> wv;
            # wrk slots (sized for wo) -> wo.
            inp = ctx.enter_context(tc.tile_pool(name="inp", bufs=8))
            psum = ctx.enter_context(tc.tile_pool(name="psum", bufs=2, space="PSUM"))
            ppool = ctx.enter_context(tc.tile_pool(name="ppool", bufs=34))
            xsp = ctx.enter_context(tc.tile_pool(name="xsp", bufs=8))
            sgp = ctx.enter_context(tc.tile_pool(name="sgp", bufs=2))
            bcp = ctx.enter_context(tc.tile_pool(name="bcp", bufs=2))
            tp = ctx.enter_context(tc.tile_pool(name="tp", bufs=4))
            ysb = ctx.enter_context(tc.tile_pool(name="ysb", bufs=2))

            # ---- input DMAs: one large transfer per tensor (small DMAs
            # waste queue slots; delivery is bandwidth-bound), need-order on
            # the sync queue; v/rv/wo inputs ride the gpsimd queue and start
            # when their reused slots free up (WAR).
            def load_all(pool_tag, dram, rows, cols, engine, nsplit=1):
                t = inp.tile([128, 8 * cols], bf16, tag=pool_tag, name=pool_tag, bufs=1)
                tv = t[:].rearrange("p (k c) -> p k c", k=8)
                dv = dram.rearrange("(k p) c -> p k c", k=8, p=128)
                for s in range(nsplit):
                    a, b = 8 * s // nsplit, 8 * (s + 1) // nsplit
                    engine.dma_start(out=tv[:, a:b, :], in_=dv[:, a:b, :])
                return [t[:, cols * k : cols * k + cols] for k in range(8)]

            wq_ch = load_all("w1", wqT, D, HD, nc.sync)
            xq_ch = load_all("xbig", xqT, D, LQ, nc.sync, nsplit=2)
            wk_ch = load_all("w2", wkT, D, HD, nc.sync)
            xk_ch = load_all("xk", xkT, D, LKP, nc.sync)
            wrk_ch = load_all("w3", wrkT, D, HD, nc.sync)
            xr_ch = load_all("xr", xrT, D, LKP, nc.sync)
            wv_ch = load_all("w2", wvT, D, HD, nc.gpsimd)
            xv_ch = load_all("xk", xvT, D, LKP, nc.gpsimd)
            wrv_ch = load_all("w1", wrvT, D, HD, nc.gpsimd)
            wo_all = inp.tile([128, 4096], bf16, tag="w3", name="wo_all", bufs=1)
            nc.gpsimd.dma_start(
                out=wo_all[:].rearrange("p (k c) -> p k c", k=4),
                in_=woT.rearrange("(k p) c -> p k c", k=4, p=128),
            )
            wo_ch = [wo_all[:, 1024 * k : 1024 * k + 1024] for k in range(4)]

            p_tiles = {}

            def emit_scores(dc, lqh, br):
                kt = kT_sb if br == 0 else rkT_sb
                qsl = slice(1024 * dc + 512 * lqh, 1024 * dc + 512 * lqh + 512)
                for m in range(NM):
                    ksl = slice(LKP * dc + 128 * m, LKP * dc + 128 * m + 128)
                    s = psum.tile([128, 1024], f32, tag="spool", name="s")
                    nc.tensor.matmul(s[:, 0:512], kt[0:64, ksl], qT_sb[0:64, qsl])
                    nc.tensor.matmul(s[:, 512:1024], kt[64:128, ksl], qT_sb[64:128, qsl])
                    p = ppool.tile([128, 1024], bf16, tag="ppool", name="p")
                    nc.scalar.activation(
                        p[:], s[:], Exp, bias=maskb_sb[:, m : m + 1], scale=SCALE
                    )
                    p_tiles[(lqh, dc, m, br)] = p

            def proj_T(dc, xch, wch, b_sb, out_sb, LL):
                # Transposed projection chunk dc -> out_sb[:, LL*dc : LL*dc+LL].
                # Chunk-outer so each k-chunk's matmuls issue as its DMA lands.
                nsl = [slice(a, min(a + 512, LL)) for a in range(0, LL, 512)]
                pss = [
                    psum.tile([128, 512], f32, tag="bank", name=f"ps{i}", bufs=4)
                    for i in range(len(nsl))
                ]
                for k in range(8):
                    for i, sl in enumerate(nsl):
                        nc.tensor.matmul(
                            pss[i][:, 0 : sl.stop - sl.start],
                            wch[k][:, 128 * dc : 128 * dc + 128],
                            xch[k][:, sl],
                            start=(k == 0),
                            stop=(k == 7),
                        )
                for i, sl in enumerate(nsl):
                    nc.vector.tensor_scalar(
                        out=out_sb[:, LL * dc + sl.start : LL * dc + sl.stop],
                        in0=pss[i][:, 0 : sl.stop - sl.start],
                        scalar1=b_sb[:, dc : dc + 1],
                        scalar2=None,
                        op0=Add,
                    )

            # ---- phase A: q/k/rk projections + lqh0 scores, dc-granular ----
            for dc in range(4):
                proj_T(dc, xq_ch, wq_ch, bq_sb, qT_sb, LQ)
                proj_T(dc, xk_ch, wk_ch, bk_sb, kT_sb, LKP)
                emit_scores(dc, 0, 0)
                proj_T(dc, xr_ch, wrk_ch, brk_sb, rkT_sb, LKP)
                emit_scores(dc, 0, 1)

            # ---- phase B: v/rv projections (natural orientation) ----
            bvv = bv_sb[:].rearrange("p (hp t c) -> p hp t c", hp=4, t=2, c=64)
            brvv = brv_sb[:].rearrange("p (hp t c) -> p hp t c", hp=4, t=2, c=64)
            for xch, wch, bview, out5 in ((xv_ch, wv_ch, bvv, v5), (xr_ch, wrv_ch, brvv, rv5)):
                for m in range(NM):
                    ps = psum.tile([128, 512], f32, tag="bank", name="psv", bufs=4)
                    for k in range(8):
                        nc.tensor.matmul(
                            ps[:, 0:512],
                            xch[k][:, 128 * m : 128 * m + 128],
                            wch[k][:],
                            start=(k == 0),
                            stop=(k == 7),
                        )
                    pv = ps[:, 0:512].rearrange("p (hp t c) -> p hp t c", hp=4, t=2, c=64)
                    nc.vector.tensor_tensor(
                        out=out5[:, m, :, :, 0:64], in0=pv[:], in1=bview[:], op=Add
                    )

            def emit_pv_chain(lqh, dc):
                # PV accumulators: 4 one-bank psum regions [65,512] keyed
                # (br, hs); freed for the next PV as soon as the copies run.
                xa = {}
                for br in range(2):
                    for hs in range(2):
                        t = psum.tile(
                            [128, 512], f32, tag="bank", name=f"xa{br}{hs}", bufs=4
                        )
                        xa[(br, hs)] = t[0:65, :]
                for br in range(2):
                    vv = v5 if br == 0 else rv5
                    for m in range(NM):
                        pt = p_tiles[(lqh, dc, m, br)]
                        for hs in range(2):
                            nc.tensor.matmul(
                                xa[(br, hs)],
                                vv[:, m, dc, hs, 0:65],
                                pt[:, 512 * hs : 512 * hs + 512],
                                start=(m == 0),
                                stop=(m == NM - 1),
                            )
                it = 2 * dc + lqh
                # Wide psum->SBUF bf16 copies packed into one tile (x rows +
                # denominator row), one den-row DMA to the DRAM bounce, one
                # [128,16]-lane reciprocal, one wide partition-broadcast back.
                # For lqh1 chains the scalar engine (idle, exp done) takes two
                # of the four copies to halve the copy-stage latency.
                xs_all = xsp.tile([65, 2048], bf16, tag="xs", name="xs_all", bufs=2)
                xs = {}
                for j, (br, hs) in enumerate(((0, 0), (1, 0), (0, 1), (1, 1))):
                    sl_ = xs_all[:, 512 * j : 512 * j + 512]
                    if lqh == 1 and j >= 2:
                        nc.scalar.copy(sl_, xa[(br, hs)])
                    else:
                        nc.vector.tensor_copy(out=sl_, in_=xa[(br, hs)])
                    xs[(br, hs)] = sl_
                nc.sync.dma_start(out=scr1[it, :], in_=xs_all[64:65, :])
                sgi = sgp.tile([128, 16], bf16, tag="sgi")
                nc.sync.dma_start(out=sgi[:], in_=scr1[it, :])
                sgo = sgp.tile([128, 16], bf16, tag="sgo")
                with nc.allow_low_precision(reason="bf16 1/denom, ~0.4% rel"):
                    nc.vector.reciprocal(sgo[:], sgi[:])
                nc.sync.dma_start(out=scr2[it, :], in_=sgo[:])
                bca = bcp.tile([64, 2048], bf16, tag="bcp", name="bca")
                nc.gpsimd.dma_start(
                    out=bca[:],
                    in_=scr2[it : it + 1, :].partition_broadcast(64)[:, 0, :],
                )
                bcs = [bca[:, 512 * j : 512 * j + 512] for j in range(4)]
                xfsl = slice(1024 * dc + 512 * lqh, 1024 * dc + 512 * lqh + 512)
                t1 = tp.tile([64, 512], bf16, tag="tp", name="t1")
                t2 = tp.tile([64, 512], bf16, tag="tp", name="t2")
                nc.vector.tensor_tensor(
                    out=t1[:], in0=xs[(0, 0)][0:64, :], in1=bcs[0][:], op=Mult
                )
                nc.vector.tensor_tensor(
                    out=t2[:], in0=xs[(1, 0)][0:64, :], in1=bcs[1][:], op=Mult
                )
                nc.vector.tensor_tensor(
                    out=xf_sb[0:64, xfsl], in0=t1[:], in1=t2[:], op=Add
                )
                t3 = tp.tile([64, 512], bf16, tag="tp", name="t3")
                t4 = tp.tile([64, 512], bf16, tag="tp", name="t4")
                nc.vector.tensor_tensor(
                    out=t3[:], in0=xs[(0, 1)][0:64, :], in1=bcs[2][:], op=Mult
                )
                nc.vector.tensor_tensor(
                    out=t4[:], in0=xs[(1, 1)][0:64, :], in1=bcs[3][:], op=Mult
                )
                thi = tp.tile([64, 512], bf16, tag="thi", name="thi", bufs=2)
                nc.vector.tensor_tensor(out=thi[:], in0=t3[:], in1=t4[:], op=Add)
                nc.gpsimd.dma_start(out=xf_sb[64:128, xfsl], in_=thi[:])

            # ---- phase C: software-pipelined so the exp engine stays fed
            # and every chain hides under the next iteration's tensor work ----
            for dc in range(4):
                emit_pv_chain(0, dc)
                emit_scores(dc, 1, 0)
                emit_scores(dc, 1, 1)
                if dc >= 1:
                    emit_pv_chain(1, dc - 1)

            def emit_y_out(reg, ot, lqh):
                # PSUM -> SBUF bf16 copy (scalar engine for the lqh0 batch --
                # idle once exp is done -- DVE for the final batch), then DMA.
                y = ysb.tile([128, 512], bf16, tag="ysb", name="y")
                if lqh == 0:
                    nc.scalar.copy(y[:], reg)
                else:
                    nc.vector.tensor_copy(out=y[:], in_=reg)
                nc.sync.dma_start(
                    out=yT[128 * ot : 128 * ot + 128, 512 * lqh : 512 * lqh + 512],
                    in_=y[:],
                )

            def emit_outproj(lqh, wide):
                if not wide:
                    for pair in range(4):
                        yt2 = psum.tile([128, 1024], f32, tag="spool", name="yt2")
                        for half in range(2):
                            ot = 2 * pair + half
                            reg = yt2[:, 512 * half : 512 * half + 512]
                            for dc in range(4):
                                nc.tensor.matmul(
                                    reg,
                                    wo_ch[dc][:, 128 * ot : 128 * ot + 128],
                                    xf_sb[:, 1024 * dc + 512 * lqh : 1024 * dc + 512 * lqh + 512],
                                    start=(dc == 0),
                                    stop=(dc == 3),
                                )
                            emit_y_out(reg, ot, lqh)
                    return
                # Wide: 8 accumulators (2 prj + 2 spool slots, 2 halves each),
                # dc-outer so only the last dim-chunk waits on the last chain.
                regs = []
                for i in range(4):
                    w2 = psum.tile([128, 512], f32, tag="bank", name=f"ywb{i}", bufs=4)
                    regs.append(w2[:])
                for i in range(2):
                    w2 = psum.tile([128, 1024], f32, tag="spool", name=f"yws{i}")
                    regs.append(w2[:, 0:512])
                    regs.append(w2[:, 512:1024])
                for dc in range(4):
                    for ot in range(8):
                        nc.tensor.matmul(
                            regs[ot],
                            wo_ch[dc][:, 128 * ot : 128 * ot + 128],
                            xf_sb[:, 1024 * dc + 512 * lqh : 1024 * dc + 512 * lqh + 512],
                            start=(dc == 0),
                            stop=(dc == 3),
                        )
                for ot in range(8):
                    emit_y_out(regs[ot], ot, lqh)

            # ---- phase D: last PV chain hides under the output projection ----
            emit_pv_chain(1, 3)
            emit_outproj(0, wide=False)
            emit_outproj(1, wide=True)

    nc.compile()
    return nc


def _get_program(lkp=LKP):
    if lkp not in _CACHE:
        _CACHE[lkp] = _build_program(lkp)
    return _CACHE[lkp]


def _cast_bf16(arr):
    import ml_dtypes

    return np.ascontiguousarray(arr.astype(ml_dtypes.bfloat16))


def _shard_inputs(inputs, lkp=LKP):
    q = np.ascontiguousarray(inputs["query"], dtype=np.float32)
    k = np.ascontiguousarray(inputs["key"], dtype=np.float32)
    v = np.ascontiguousarray(inputs["value"], dtype=np.float32)
    wr = np.ascontiguousarray(inputs["weak_rela"], dtype=np.float32)
    mask = np.asarray(inputs["mask"])

    in_maps = []
    for c in range(N_CORES):
        b, hh = divmod(c, 2)
        hsl = slice(HD * hh, HD * hh + HD)
        idx = np.nonzero(mask[b, 0])[0]
        nv = len(idx)
        assert nv <= lkp
        pidx = np.concatenate([idx, np.zeros(lkp - nv, dtype=idx.dtype)])
        bias = np.full(lkp, -1.0e9, np.float32)
        bias[:nv] = 0.0
        mb = np.ascontiguousarray(bias.reshape(lkp // 128, 128).T)
        kc, vc, wrc = k[b][pidx], v[b][pidx], wr[b][pidx]
        m = {
            "xqT": _cast_bf16(q[b].T),
            "xkT": _cast_bf16(kc.T),
            "xrT": _cast_bf16(wrc.T),
            "xvT": _cast_bf16(vc.T),
            "wqT": _cast_bf16(np.asarray(inputs["Wq"])[hsl, :].T),
            "wkT": _cast_bf16(np.asarray(inputs["Wk"])[hsl, :].T),
            "wrkT": _cast_bf16(np.asarray(inputs["Wrk"])[hsl, :].T),
            "wvT": _cast_bf16(np.asarray(inputs["Wv"])[hsl, :].T),
            "wrvT": _cast_bf16(np.asarray(inputs["Wrv"])[hsl, :].T),
            "woT": _cast_bf16(np.asarray(inputs["Wo"])[:, hsl].T),
            "bq_pc": np.asarray(inputs["bq"][hsl]).reshape(4, 128).T.astype(np.float32),
            "bk_pc": np.asarray(inputs["bk"][hsl]).reshape(4, 128).T.astype(np.float32),
            "brk_pc": np.asarray(inputs["brk"][hsl])
            .reshape(4, 128)
            .T.astype(np.float32),
            "bv_bc": np.broadcast_to(inputs["bv"][hsl], (128, HD)).astype(np.float32),
            "brv_bc": np.broadcast_to(inputs["brv"][hsl], (128, HD)).astype(np.float32),
            "maskb": mb,
        }
        in_maps.append({k2: np.ascontiguousarray(v2) for k2, v2 in m.items()})
    return in_maps


def run_on_hw(inputs, trace=False, **kw):
    from concourse.bass_utils import run_bass_kernel_spmd

    mask = np.asarray(inputs["mask"])
    max_valid = max(int(mask[b, 0].sum()) for b in range(B))
    lkp = max(LKP, ((max_valid + 127) // 128) * 128)
    nc = _get_program(lkp)
    in_maps = _shard_inputs(inputs, lkp)
    res = run_bass_kernel_spmd(
        nc, in_maps, core_ids=list(range(N_CORES)), trace=trace, **kw
    )
    bo = np.asarray(inputs["bo"], dtype=np.float32)
    outs = []
    for b in range(B):
        yt = res.results[2 * b]["yT"].astype(np.float32) + res.results[
            2 * b + 1
        ]["yT"].astype(np.float32)
        outs.append(yt.T + bo)
    out = np.stack(outs).astype(np.float32)
    return out, res


def kernel(**inputs):
    out, _ = run_on_hw(inputs)
    return out
